# revision 19
# baseline (speedup 1.0000x reference)
"""DeepSeek-V2-Lite MoE layer on 8 Trainium2 NeuronCores.

Strategy (expert-parallel, per the sharding hint):
  - Host computes the gate (256x64 matmul + softmax + top-6) in fp32 numpy and
    builds per-core token batches ("all-to-all" realized host-side under the
    full-IO contract).
  - Each core owns 8 routed experts (expert-axis sharding) and a 1/8 slice of
    the shared expert intermediate dim (tensor-parallel).
  - Routed expert weights are streamed as fp8-e3m4 (scaled x64 into the e3m4
    range; tokens are pre-scaled by 1/64 so gate/up PSUM values are exact, and
    the down-projection's x64 is divided out in the host combine). This halves
    the dominant weight DMA vs bf16: ~69 MB/core, the HBM roofline.
  - All matmuls are weights-stationary: each [128,128] weight tile is the
    (fast-weight-load) stationary operand and the token batch streams as the
    moving operand, so PE time ~ #tiles * ~60cy, well under the DMA floor, and
    no PE transposes are needed anywhere (gate/up output lands FFN-major,
    exactly the down matmul's moving layout; down output lands HIDDEN-major
    and the host undoes it in the combine).
  - PSUM discipline: one accumulation group per 2KB bank (start=True clears
    has_written bank-wide), and g/u/down groups live in separate pools so a
    drain (ScalarE/VectorE read) never touches a bank the PE is writing.
  - Per-token routing weights are applied in the host combine (tokens live on
    the free axis on device, so the fold-in is cheapest on host).
"""

import os
import numpy as np
import ml_dtypes

BF16 = ml_dtypes.bfloat16
F8E3 = ml_dtypes.float8_e3m4

HIDDEN = 2048
FFN = 1408
N_EXPERTS = 64
TOP_K = 6
SHARED_FFN = 2816          # 2 shared experts * FFN
T = 256
N_CORES = 8
EPC = N_EXPERTS // N_CORES   # experts per core = 8
SFS = SHARED_FFN // N_CORES  # shared-FFN slice per core = 352

N_KH = HIDDEN // 128   # 16 hidden-dim chunks
N_MF = FFN // 128      # 11 ffn tiles (gate/up pairs; also down k-chunks)
N_MH = HIDDEN // 128   # 16 down-output tiles
GU_FREE = N_MF * 2 * N_KH * 128   # 45056 B/partition of gate+up fp8 weights
DN_FREE = N_MH * N_MF * 128       # 22528 B/partition of down fp8 weights
S8 = 64.0                         # fp8 weight scale (power of 2: exact folds)
F8MAX = 15.5                      # e3m4 max normal
N_SF = 3                          # shared ffn-slice tiles (352 -> 3x128 padded)

# gate+up weight chunk boundaries (in 128-col ffn pair-tiles) for DMA overlap;
# first chunk smallest so each expert's first matmul (and the next slot's
# buffer-free) comes earliest
GU_CH = [(0, 1), (1, 2), (3, 2), (5, 3), (8, 3)]
# extra buffer on the first gate tag: it fronts each expert's SP chain, so a
# third buffer lets one more expert's stream start wait-free
GU_BUFS = [3, 2, 2, 2, 2]
# down weight chunk boundaries (in 128-col hidden tiles)
DN_CH = [(0, 4), (4, 4), (8, 4), (12, 4)]

_PROGRAM_CACHE = {}
LAST_RESULTS = None


def _route(x, gate_w):
    """fp32 softmax top-k routing, matching jax.lax.top_k tie-breaking
    (stable sort -> lowest index wins ties)."""
    logits = x @ gate_w.T                      # [T, E] fp32
    m = logits.max(axis=-1, keepdims=True)
    e = np.exp(logits - m)
    scores = e / e.sum(axis=-1, keepdims=True)
    ids = np.argsort(-scores, axis=-1, kind="stable")[:, :TOP_K]
    w = np.take_along_axis(scores, ids, axis=-1)
    w = w / (w.sum(axis=-1, keepdims=True) + 1e-20)
    return ids, w.astype(np.float32)


def _build_program(C):
    import concourse.bass as bass
    import concourse.bacc as bacc
    import concourse.mybir as mybir
    import concourse.tile as tile
    from contextlib import ExitStack

    f32 = mybir.dt.float32
    bf16 = mybir.dt.bfloat16
    f8e3 = mybir.dt.float8e3
    SILU = mybir.ActivationFunctionType.Silu

    # Bacc (not plain Bass): its compile pipeline splits multi-wait
    # instructions into the 1-wait-per-instruction form TRN2 requires.
    nc = bacc.Bacc(None)

    d_w8 = nc.dram_tensor("w8", [EPC, 128, GU_FREE + DN_FREE], f8e3,
                          kind="ExternalInput")
    d_xt = nc.dram_tensor("xt", [EPC, 128, N_KH * C], bf16, kind="ExternalInput")
    d_xsh = nc.dram_tensor("xsh", [128, N_KH * 256], bf16, kind="ExternalInput")
    d_wsgu = nc.dram_tensor("wsgu", [128, 2 * N_SF * N_KH * 128], bf16,
                            kind="ExternalInput")
    d_wsd = nc.dram_tensor("wsd", [128, N_MH * N_SF * 128], bf16,
                           kind="ExternalInput")
    d_yrt = nc.dram_tensor("yrt", [EPC, 128, N_MH * C], bf16, kind="ExternalOutput")
    d_ysh = nc.dram_tensor("ysh", [128, N_MH * 256], bf16, kind="ExternalOutput")

    with tile.TileContext(nc) as tc, ExitStack() as ctx:
        p_w = ctx.enter_context(tc.tile_pool(name="w8", bufs=2))
        p_xt = ctx.enter_context(tc.tile_pool(name="xt", bufs=EPC))
        p_act = ctx.enter_context(tc.tile_pool(name="act", bufs=2))
        p_gs = ctx.enter_context(tc.tile_pool(name="gs", bufs=3))
        p_out = ctx.enter_context(tc.tile_pool(name="out", bufs=2))
        p_sw = ctx.enter_context(tc.tile_pool(name="sw", bufs=1))
        p_sact = ctx.enter_context(tc.tile_pool(name="sact", bufs=1))
        p_sout = ctx.enter_context(tc.tile_pool(name="sout", bufs=1))
        # one accumulation group per bank-tile; g/u/y in separate pools so no
        # bank ever sees a second group's start= or a drain during PE writes
        ps_g = ctx.enter_context(tc.tile_pool(name="ps_g", bufs=3, space="PSUM"))
        ps_u = ctx.enter_context(tc.tile_pool(name="ps_u", bufs=3, space="PSUM"))
        ps_y = ctx.enter_context(tc.tile_pool(name="ps_y", bufs=2, space="PSUM"))

        # token batches for every slot prefetch on the ACT ring at t=0 (their
        # buffers are all distinct, so the issues carry no semaphore waits);
        # slot 0's batch and the shared gate tensors go first so the PE's
        # ramp work (slot 0 + the three shared gate pieces) is never starved
        xt_tiles = [p_xt.tile([128, N_KH * C], bf16, tag="xt", name=f"xt{s}")
                    for s in range(EPC)]
        nc.scalar.dma_start(out=xt_tiles[0][:], in_=d_xt[0])
        xsh = p_sw.tile([128, N_KH * 256], bf16, tag="xsh")
        nc.scalar.dma_start(out=xsh[:], in_=d_xsh[:])
        wsg = p_sw.tile([128, 2 * N_SF * N_KH * 128], bf16, tag="wsgu")
        nc.scalar.dma_start(out=wsg[:], in_=d_wsgu[:])
        for s in range(1, EPC):
            nc.scalar.dma_start(out=xt_tiles[s][:], in_=d_xt[s])
        wsd = p_sw.tile([128, N_MH * N_SF * 128], bf16, tag="wsd")
        nc.scalar.dma_start(out=wsd[:], in_=d_wsd[:])
        act_sh = p_sact.tile([128, N_SF * 256], bf16, tag="act")

        def routed_slot(s):
            # gate/up weights stream on the SP ring, gated only by early-period
            # reads so the issue queue never serializes on late-period events;
            # down weights ride the ACT ring, whose other issues are wait-free
            xt = xt_tiles[s]
            gu_tiles = []
            for i, (m0, nm) in enumerate(GU_CH):
                wt = p_w.tile([128, nm * 2 * N_KH * 128], f8e3, tag=f"gu{i}",
                              bufs=GU_BUFS[i])
                nc.sync.dma_start(
                    out=wt[:], in_=d_w8[s, :, m0 * 4096:(m0 + nm) * 4096]
                )
                gu_tiles.append((m0, nm, wt))
            dn_tiles = []
            for i, (h0, nh) in enumerate(DN_CH):
                wt = p_w.tile([128, nh * N_MF * 128], f8e3, tag=f"dn{i}")
                nc.sync.dma_start(
                    out=wt[:],
                    in_=d_w8[s, :, GU_FREE + h0 * 1408:GU_FREE + (h0 + nh) * 1408],
                )
                dn_tiles.append((h0, nh, wt))
            # ---- gate+up: fp8 weight tiles stationary, tokens moving ----
            act = p_act.tile([128, N_MF * C], bf16, tag="act")
            for (m0, nm, wt) in gu_tiles:
                for mi in range(nm):
                    m = m0 + mi
                    pg = ps_g.tile([128, 512], f32, tag="pg")
                    pu = ps_u.tile([128, 512], f32, tag="pu")
                    for k in range(N_KH):
                        rhs = xt[:, k * C:(k + 1) * C]
                        nc.tensor.matmul(
                            pg[:, :C],
                            wt[:, mi * 4096 + k * 128: mi * 4096 + (k + 1) * 128],
                            rhs, start=(k == 0), stop=(k == N_KH - 1),
                        )
                        nc.tensor.matmul(
                            pu[:, :C],
                            wt[:, mi * 4096 + 2048 + k * 128: mi * 4096 + 2048 + (k + 1) * 128],
                            rhs, start=(k == 0), stop=(k == N_KH - 1),
                        )
                    gs = p_gs.tile([128, C], f32, tag="gs")
                    nc.scalar.activation(gs[:], pg[:, :C], SILU)
                    nc.vector.tensor_mul(act[:, m * C:(m + 1) * C], gs[:], pu[:, :C])
            # ---- down: fp8 weight tiles stationary, act moving ----
            out_sb = p_out.tile([128, N_MH * C], bf16, tag="out")
            for (h0, nh, wt) in dn_tiles:
                for hi in range(nh):
                    hm = h0 + hi
                    py = ps_y.tile([128, 512], f32, tag="py")
                    for j in range(N_MF):
                        nc.tensor.matmul(
                            py[:, :C],
                            wt[:, hi * 1408 + j * 128: hi * 1408 + (j + 1) * 128],
                            act[:, j * C:(j + 1) * C],
                            start=(j == 0), stop=(j == N_MF - 1),
                        )
                    nc.vector.tensor_copy(out_sb[:, hm * C:(hm + 1) * C], py[:, :C])
            nc.scalar.dma_start(out=d_yrt[s], in_=out_sb[:])

        # the shared compute is emitted in small pieces: the gate pieces fill
        # the PE during the DMA ramp, the down pieces slot between later
        # experts (a single 16us shared block creates a PE-lag bubble that
        # locks the whole pipeline into an issue-cluster limit cycle)
        def shared_gate(m):
            # gate+up over the 352-wide slice: one zero-padded 128-col pair
            pg = ps_g.tile([128, 512], f32, tag="pg")
            pu = ps_u.tile([128, 512], f32, tag="pu")
            for k in range(N_KH):
                rhs = xsh[:, k * 256:(k + 1) * 256]
                nc.tensor.matmul(
                    pg[:, :256], wsg[:, (m * N_KH + k) * 128:(m * N_KH + k + 1) * 128],
                    rhs, start=(k == 0), stop=(k == N_KH - 1),
                )
                nc.tensor.matmul(
                    pu[:, :256],
                    wsg[:, ((N_SF + m) * N_KH + k) * 128:((N_SF + m) * N_KH + k + 1) * 128],
                    rhs, start=(k == 0), stop=(k == N_KH - 1),
                )
            gs = p_gs.tile([128, 256], f32, tag="sgs", bufs=2)
            nc.scalar.activation(gs[:], pg[:, :256], SILU)
            nc.vector.tensor_mul(act_sh[:, m * 256:(m + 1) * 256], gs[:], pu[:, :256])

        def shared_down(ci):
            out_sh = p_sout.tile([128, 4 * 256], bf16, tag="out", bufs=2)
            for hi in range(4):
                hm = ci * 4 + hi
                py = ps_y.tile([128, 512], f32, tag="py")
                for j in range(N_SF):
                    nc.tensor.matmul(
                        py[:, :256],
                        wsd[:, (hm * N_SF + j) * 128:(hm * N_SF + j + 1) * 128],
                        act_sh[:, j * 256:(j + 1) * 256],
                        start=(j == 0), stop=(j == N_SF - 1),
                    )
                nc.vector.tensor_copy(out_sh[:, hi * 256:(hi + 1) * 256], py[:, :256])
            nc.scalar.dma_start(out=d_ysh[:, ci * 1024:(ci + 1) * 1024], in_=out_sh[:])

        routed_slot(0)
        shared_gate(0)
        shared_gate(1)
        shared_gate(2)
        routed_slot(1)
        routed_slot(2)
        routed_slot(3)
        shared_down(0)
        routed_slot(4)
        shared_down(1)
        routed_slot(5)
        shared_down(2)
        routed_slot(6)
        shared_down(3)
        routed_slot(7)

    if not nc.is_finalized():
        nc.finalize()
    return nc


def _sbufize(a, kdim):
    """[K*128, N] -> [128, K*N] SBUF layout (K-chunks along free dim)."""
    K = a.shape[0] // 128
    return np.ascontiguousarray(
        a.reshape(K, 128, -1).transpose(1, 0, 2).reshape(128, -1)
    )


def _quant8(w):
    """scale x64 into e3m4 range and quantize (RNE, clipped to max normal)."""
    return np.clip(w * S8, -F8MAX, F8MAX).astype(F8E3)


def kernel(hidden_states, gate_w, w_gate_up, w_down, ws_gate_up, ws_down):
    global LAST_RESULTS
    x = np.asarray(hidden_states, dtype=np.float32).reshape(T, HIDDEN)
    gate_w = np.asarray(gate_w, dtype=np.float32)

    ids, tw = _route(x, gate_w)

    # per-expert token lists + positions
    lists = [[] for _ in range(N_EXPERTS)]
    pos = np.zeros((T, TOP_K), dtype=np.int64)
    for t in range(T):
        for i in range(TOP_K):
            e = ids[t, i]
            pos[t, i] = len(lists[e])
            lists[e].append(t)
    maxload = max(len(l) for l in lists)
    C = max(32, -(-maxload // 16) * 16)
    assert C <= 128, f"expert overload {maxload}: splitting not implemented"

    xT = np.ascontiguousarray(x.T)  # fp32 [H, T]

    w_gate_up = np.asarray(w_gate_up, dtype=np.float32)
    w_down = np.asarray(w_down, dtype=np.float32)
    ws_gate_up = np.asarray(ws_gate_up, dtype=np.float32)
    ws_down = np.asarray(ws_down, dtype=np.float32)

    # ---- fp8 weight quantization + stationary-tile layout (all experts) ----
    # gate/up tile (pair m, which, k): [h_sub p, f_col c] = W[w*FFN+m*128+c, k*128+p]
    gu8 = _quant8(w_gate_up)  # [E, 2816, 2048]
    gu_l = np.ascontiguousarray(
        gu8.reshape(N_EXPERTS, 2, N_MF, 128, N_KH, 128)
        .transpose(0, 5, 2, 1, 4, 3)
        .reshape(N_EXPERTS, 128, GU_FREE)
    )
    # down tile (hm, j): [f_sub p, h_col c] = W[hm*128+c, j*128+p]
    dn8 = _quant8(w_down)  # [E, 2048, 1408]
    dn_l = np.ascontiguousarray(
        dn8.reshape(N_EXPERTS, N_MH, 128, N_MF, 128)
        .transpose(0, 4, 1, 3, 2)
        .reshape(N_EXPERTS, 128, DN_FREE)
    )
    w8_all = np.concatenate([gu_l, dn_l], axis=2)  # [E, 128, GU_FREE+DN_FREE]

    xT_s = xT / S8  # tokens pre-scaled so gate/up PSUM is exact (power of 2)

    in_maps = []
    for c in range(N_CORES):
        xts = np.zeros((EPC, 128, N_KH * C), dtype=BF16)
        for s in range(EPC):
            e = c * EPC + s
            toks = lists[e]
            n = len(toks)
            if n:
                xte = np.zeros((HIDDEN, C), dtype=np.float32)
                xte[:, :n] = xT_s[:, toks]
                xts[s] = _sbufize(xte.astype(BF16), N_KH)
        # shared expert slice (tensor-parallel on intermediate dim), padded to
        # 3x128 cols of zeros beyond 352 so every stationary tile is 128 wide
        g_pad = np.zeros((N_SF * 128, HIDDEN), dtype=np.float32)
        u_pad = np.zeros((N_SF * 128, HIDDEN), dtype=np.float32)
        g_pad[:SFS] = ws_gate_up[c * SFS:(c + 1) * SFS]
        u_pad[:SFS] = ws_gate_up[SHARED_FFN + c * SFS: SHARED_FFN + (c + 1) * SFS]
        gu_sh = np.concatenate([g_pad, u_pad], axis=0)       # [768, H] (f, h)
        # tile (mt, k): [h_sub p, f_col c] = gu_sh[mt*128+c, k*128+p]
        wsgu = np.ascontiguousarray(
            gu_sh.reshape(2 * N_SF, 128, N_KH, 128)
            .transpose(3, 0, 2, 1)
            .reshape(128, 2 * N_SF * N_KH * 128)
            .astype(BF16)
        )
        wsd_pad = np.zeros((N_SF * 128, HIDDEN), dtype=np.float32)
        wsd_pad[:SFS] = ws_down[:, c * SFS:(c + 1) * SFS].T  # [384, H] (f, h)
        # tile (hm, j): [f_sub p, h_col c] = wsd_pad[j*128+p, hm*128+c]
        wsd = np.ascontiguousarray(
            wsd_pad.reshape(N_SF, 128, N_MH, 128)
            .transpose(1, 2, 0, 3)
            .reshape(128, N_MH * N_SF * 128)
            .astype(BF16)
        )
        xsh = _sbufize(xT.astype(BF16), N_KH)               # [128, 16*256]
        in_maps.append({
            "w8": w8_all[c * EPC:(c + 1) * EPC], "xt": xts,
            "xsh": xsh, "wsgu": wsgu, "wsd": wsd,
        })

    if C not in _PROGRAM_CACHE:
        _PROGRAM_CACHE[C] = _build_program(C)
    nc = _PROGRAM_CACHE[C]

    from concourse.bass_utils import run_bass_kernel_spmd
    res = run_bass_kernel_spmd(
        nc, in_maps, list(range(N_CORES)),
        trace=bool(os.environ.get("MOE_KERNEL_TRACE")),
    )
    LAST_RESULTS = res

    # ---- combine: weighted gather-sum of routed rows + shared partials ----
    # yrt[s] is [128, N_MH*C] with y[e, tok c, hm*128+p] = buf[p, hm*C + c]
    y_all = np.stack([
        r["yrt"].astype(np.float32).reshape(EPC, 128, N_MH, C)
        for r in res.results
    ])                                                      # [8, EPC, 128, 16, C]
    y_flat = y_all.transpose(0, 1, 4, 3, 2).reshape(N_EXPERTS * C, HIDDEN)
    G = ids * C + pos                                       # [T, 6]
    routed = (y_flat[G] * tw[:, :, None]).sum(axis=1) / S8
    # ysh is [128, N_MH*256] with y[tok, hm*128+p] = buf[p, hm*256 + tok]
    shared = np.sum([
        r["ysh"].astype(np.float32).reshape(128, N_MH, 256).transpose(2, 1, 0)
        for r in res.results
    ], axis=0).reshape(T, HIDDEN)
    out = routed + shared
    return out.reshape(1, T, HIDDEN).astype(np.float32)


# revision 20
# speedup vs baseline: 1.0130x; 1.0130x over previous
"""DeepSeek-V2-Lite MoE layer on 8 Trainium2 NeuronCores.

Strategy (expert-parallel, per the sharding hint):
  - Host computes the gate (256x64 matmul + softmax + top-6) in fp32 numpy and
    builds per-core token batches ("all-to-all" realized host-side under the
    full-IO contract).
  - Each core owns 8 routed experts (expert-axis sharding) and a 1/8 slice of
    the shared expert intermediate dim (tensor-parallel).
  - Routed expert weights are streamed as fp8-e3m4 (scaled x64 into the e3m4
    range; tokens are pre-scaled by 1/64 so gate/up PSUM values are exact, and
    the down-projection's x64 is divided out in the host combine). This halves
    the dominant weight DMA vs bf16: ~69 MB/core, the HBM roofline.
  - All matmuls are weights-stationary: each [128,128] weight tile is the
    (fast-weight-load) stationary operand and the token batch streams as the
    moving operand, so PE time ~ #tiles * ~60cy, well under the DMA floor, and
    no PE transposes are needed anywhere (gate/up output lands FFN-major,
    exactly the down matmul's moving layout; down output lands HIDDEN-major
    and the host undoes it in the combine).
  - PSUM discipline: one accumulation group per 2KB bank (start=True clears
    has_written bank-wide), and g/u/down groups live in separate pools so a
    drain (ScalarE/VectorE read) never touches a bank the PE is writing.
  - Per-token routing weights are applied in the host combine (tokens live on
    the free axis on device, so the fold-in is cheapest on host).
"""

import os
import numpy as np
import ml_dtypes

BF16 = ml_dtypes.bfloat16
F8E3 = ml_dtypes.float8_e3m4

HIDDEN = 2048
FFN = 1408
N_EXPERTS = 64
TOP_K = 6
SHARED_FFN = 2816          # 2 shared experts * FFN
T = 256
N_CORES = 8
EPC = N_EXPERTS // N_CORES   # experts per core = 8
SFS = SHARED_FFN // N_CORES  # shared-FFN slice per core = 352

N_KH = HIDDEN // 128   # 16 hidden-dim chunks
N_MF = FFN // 128      # 11 ffn tiles (gate/up pairs; also down k-chunks)
N_MH = HIDDEN // 128   # 16 down-output tiles
GU_FREE = N_MF * 2 * N_KH * 128   # 45056 B/partition of gate+up fp8 weights
DN_FREE = N_MH * N_MF * 128       # 22528 B/partition of down fp8 weights
S8 = 64.0                         # fp8 weight scale (power of 2: exact folds)
F8MAX = 15.5                      # e3m4 max normal
N_SF = 3                          # shared ffn-slice tiles (352 -> 3x128 padded)

# gate+up weight chunk boundaries (in 128-col ffn pair-tiles) for DMA overlap;
# first chunk smallest so each expert's first matmul (and the next slot's
# buffer-free) comes earliest
GU_CH = [(0, 1), (1, 2), (3, 2), (5, 3), (8, 3)]
# extra buffer on the first gate tag: it fronts each expert's SP chain, so a
# third buffer lets one more expert's stream start wait-free
GU_BUFS = [3, 2, 2, 2, 2]
# down weight chunk boundaries (in 128-col hidden tiles)
DN_CH = [(0, 4), (4, 4), (8, 4), (12, 4)]

_PROGRAM_CACHE = {}
LAST_RESULTS = None


def _route(x, gate_w):
    """fp32 softmax top-k routing, matching jax.lax.top_k tie-breaking
    (stable sort -> lowest index wins ties)."""
    logits = x @ gate_w.T                      # [T, E] fp32
    m = logits.max(axis=-1, keepdims=True)
    e = np.exp(logits - m)
    scores = e / e.sum(axis=-1, keepdims=True)
    ids = np.argsort(-scores, axis=-1, kind="stable")[:, :TOP_K]
    w = np.take_along_axis(scores, ids, axis=-1)
    w = w / (w.sum(axis=-1, keepdims=True) + 1e-20)
    return ids, w.astype(np.float32)


def _build_program(C):
    import concourse.bass as bass
    import concourse.bacc as bacc
    import concourse.mybir as mybir
    import concourse.tile as tile
    from contextlib import ExitStack

    f32 = mybir.dt.float32
    bf16 = mybir.dt.bfloat16
    f8e3 = mybir.dt.float8e3
    SILU = mybir.ActivationFunctionType.Silu

    # Bacc (not plain Bass): its compile pipeline splits multi-wait
    # instructions into the 1-wait-per-instruction form TRN2 requires.
    nc = bacc.Bacc(None)

    d_w8 = nc.dram_tensor("w8", [EPC, 128, GU_FREE + DN_FREE], f8e3,
                          kind="ExternalInput")
    d_xt = nc.dram_tensor("xt", [EPC, 128, N_KH * C], bf16, kind="ExternalInput")
    d_xsh = nc.dram_tensor("xsh", [128, N_KH * 256], bf16, kind="ExternalInput")
    d_wsgu = nc.dram_tensor("wsgu", [128, 2 * N_SF * N_KH * 128], bf16,
                            kind="ExternalInput")
    d_wsd = nc.dram_tensor("wsd", [128, N_MH * N_SF * 128], bf16,
                           kind="ExternalInput")
    d_yrt = nc.dram_tensor("yrt", [EPC, 128, N_MH * C], bf16, kind="ExternalOutput")
    d_ysh = nc.dram_tensor("ysh", [128, N_MH * 256], bf16, kind="ExternalOutput")

    with tile.TileContext(nc) as tc, ExitStack() as ctx:
        p_w = ctx.enter_context(tc.tile_pool(name="w8", bufs=2))
        p_xt = ctx.enter_context(tc.tile_pool(name="xt", bufs=EPC))
        p_act = ctx.enter_context(tc.tile_pool(name="act", bufs=2))
        p_gs = ctx.enter_context(tc.tile_pool(name="gs", bufs=3))
        p_out = ctx.enter_context(tc.tile_pool(name="out", bufs=2))
        p_sw = ctx.enter_context(tc.tile_pool(name="sw", bufs=1))
        p_sact = ctx.enter_context(tc.tile_pool(name="sact", bufs=1))
        p_sout = ctx.enter_context(tc.tile_pool(name="sout", bufs=1))
        # one accumulation group per bank-tile; g/u/y in separate pools so no
        # bank ever sees a second group's start= or a drain during PE writes
        ps_g = ctx.enter_context(tc.tile_pool(name="ps_g", bufs=3, space="PSUM"))
        ps_u = ctx.enter_context(tc.tile_pool(name="ps_u", bufs=3, space="PSUM"))
        ps_y = ctx.enter_context(tc.tile_pool(name="ps_y", bufs=2, space="PSUM"))

        # token batches for every slot prefetch on the ACT ring at t=0 (their
        # buffers are all distinct, so the issues carry no semaphore waits);
        # slot 0's batch and the shared gate tensors go first so the PE's
        # ramp work (slot 0 + the three shared gate pieces) is never starved
        xt_tiles = [p_xt.tile([128, N_KH * C], bf16, tag="xt", name=f"xt{s}")
                    for s in range(EPC)]
        nc.scalar.dma_start(out=xt_tiles[0][:], in_=d_xt[0])
        xsh = p_sw.tile([128, N_KH * 256], bf16, tag="xsh")
        nc.scalar.dma_start(out=xsh[:], in_=d_xsh[:])
        wsg = p_sw.tile([128, 2 * N_SF * N_KH * 128], bf16, tag="wsgu")
        nc.scalar.dma_start(out=wsg[:], in_=d_wsgu[:])
        for s in range(1, EPC):
            nc.scalar.dma_start(out=xt_tiles[s][:], in_=d_xt[s])
        wsd = p_sw.tile([128, N_MH * N_SF * 128], bf16, tag="wsd")
        nc.scalar.dma_start(out=wsd[:], in_=d_wsd[:])
        act_sh = p_sact.tile([128, N_SF * 256], bf16, tag="act")

        def routed_slot(s):
            # gate/up weights stream on the SP ring, gated only by early-period
            # reads so the issue queue never serializes on late-period events;
            # down weights ride the ACT ring, whose other issues are wait-free
            xt = xt_tiles[s]
            gu_tiles = []
            for i, (m0, nm) in enumerate(GU_CH):
                wt = p_w.tile([128, nm * 2 * N_KH * 128], f8e3, tag=f"gu{i}",
                              bufs=GU_BUFS[i])
                nc.sync.dma_start(
                    out=wt[:], in_=d_w8[s, :, m0 * 4096:(m0 + nm) * 4096]
                )
                gu_tiles.append((m0, nm, wt))
            dn_tiles = []
            for i, (h0, nh) in enumerate(DN_CH):
                wt = p_w.tile([128, nh * N_MF * 128], f8e3, tag=f"dn{i}")
                nc.sync.dma_start(
                    out=wt[:],
                    in_=d_w8[s, :, GU_FREE + h0 * 1408:GU_FREE + (h0 + nh) * 1408],
                )
                dn_tiles.append((h0, nh, wt))
            # ---- gate+up: fp8 weight tiles stationary, tokens moving ----
            act = p_act.tile([128, N_MF * C], bf16, tag="act")
            for (m0, nm, wt) in gu_tiles:
                for mi in range(nm):
                    m = m0 + mi
                    pg = ps_g.tile([128, 512], f32, tag="pg")
                    pu = ps_u.tile([128, 512], f32, tag="pu")
                    for k in range(N_KH):
                        rhs = xt[:, k * C:(k + 1) * C]
                        nc.tensor.matmul(
                            pg[:, :C],
                            wt[:, mi * 4096 + k * 128: mi * 4096 + (k + 1) * 128],
                            rhs, start=(k == 0), stop=(k == N_KH - 1),
                        )
                        nc.tensor.matmul(
                            pu[:, :C],
                            wt[:, mi * 4096 + 2048 + k * 128: mi * 4096 + 2048 + (k + 1) * 128],
                            rhs, start=(k == 0), stop=(k == N_KH - 1),
                        )
                    gs = p_gs.tile([128, C], f32, tag="gs")
                    nc.scalar.activation(gs[:], pg[:, :C], SILU)
                    nc.vector.tensor_mul(act[:, m * C:(m + 1) * C], gs[:], pu[:, :C])
            # ---- down: fp8 weight tiles stationary, act moving ----
            out_sb = p_out.tile([128, N_MH * C], bf16, tag="out")
            for (h0, nh, wt) in dn_tiles:
                for hi in range(nh):
                    hm = h0 + hi
                    py = ps_y.tile([128, 512], f32, tag="py")
                    for j in range(N_MF):
                        nc.tensor.matmul(
                            py[:, :C],
                            wt[:, hi * 1408 + j * 128: hi * 1408 + (j + 1) * 128],
                            act[:, j * C:(j + 1) * C],
                            start=(j == 0), stop=(j == N_MF - 1),
                        )
                    nc.vector.tensor_copy(out_sb[:, hm * C:(hm + 1) * C], py[:, :C])
            nc.scalar.dma_start(out=d_yrt[s], in_=out_sb[:])

        # the shared compute is emitted in small pieces: the gate pieces fill
        # the PE during the DMA ramp, the down pieces slot between later
        # experts (a single 16us shared block creates a PE-lag bubble that
        # locks the whole pipeline into an issue-cluster limit cycle)
        def shared_gate(m):
            # gate+up over the 352-wide slice: one zero-padded 128-col pair
            pg = ps_g.tile([128, 512], f32, tag="pg")
            pu = ps_u.tile([128, 512], f32, tag="pu")
            for k in range(N_KH):
                rhs = xsh[:, k * 256:(k + 1) * 256]
                nc.tensor.matmul(
                    pg[:, :256], wsg[:, (m * N_KH + k) * 128:(m * N_KH + k + 1) * 128],
                    rhs, start=(k == 0), stop=(k == N_KH - 1),
                )
                nc.tensor.matmul(
                    pu[:, :256],
                    wsg[:, ((N_SF + m) * N_KH + k) * 128:((N_SF + m) * N_KH + k + 1) * 128],
                    rhs, start=(k == 0), stop=(k == N_KH - 1),
                )
            gs = p_gs.tile([128, 256], f32, tag="sgs", bufs=2)
            nc.scalar.activation(gs[:], pg[:, :256], SILU)
            nc.vector.tensor_mul(act_sh[:, m * 256:(m + 1) * 256], gs[:], pu[:, :256])

        def shared_down(ci):
            out_sh = p_sout.tile([128, 4 * 256], bf16, tag="out", bufs=2)
            for hi in range(4):
                hm = ci * 4 + hi
                py = ps_y.tile([128, 512], f32, tag="py")
                for j in range(N_SF):
                    nc.tensor.matmul(
                        py[:, :256],
                        wsd[:, (hm * N_SF + j) * 128:(hm * N_SF + j + 1) * 128],
                        act_sh[:, j * 256:(j + 1) * 256],
                        start=(j == 0), stop=(j == N_SF - 1),
                    )
                nc.vector.tensor_copy(out_sh[:, hi * 256:(hi + 1) * 256], py[:, :256])
            nc.scalar.dma_start(out=d_ysh[:, ci * 1024:(ci + 1) * 1024], in_=out_sh[:])

        routed_slot(0)
        shared_gate(0)
        shared_gate(1)
        shared_gate(2)
        routed_slot(1)
        routed_slot(2)
        routed_slot(3)
        shared_down(0)
        routed_slot(4)
        shared_down(1)
        routed_slot(5)
        shared_down(2)
        routed_slot(6)
        shared_down(3)
        routed_slot(7)

    if not nc.is_finalized():
        nc.finalize()
    return nc


def _sbufize(a, kdim):
    """[K*128, N] -> [128, K*N] SBUF layout (K-chunks along free dim)."""
    K = a.shape[0] // 128
    return np.ascontiguousarray(
        a.reshape(K, 128, -1).transpose(1, 0, 2).reshape(128, -1)
    )


def _quant8(w):
    """scale x64 into e3m4 range and quantize (RNE, clipped to max normal)."""
    return np.clip(w * S8, -F8MAX, F8MAX).astype(F8E3)


def kernel(hidden_states, gate_w, w_gate_up, w_down, ws_gate_up, ws_down):
    global LAST_RESULTS
    x = np.asarray(hidden_states, dtype=np.float32).reshape(T, HIDDEN)
    gate_w = np.asarray(gate_w, dtype=np.float32)

    ids, tw = _route(x, gate_w)

    # per-expert token lists + positions
    lists = [[] for _ in range(N_EXPERTS)]
    pos = np.zeros((T, TOP_K), dtype=np.int64)
    for t in range(T):
        for i in range(TOP_K):
            e = ids[t, i]
            pos[t, i] = len(lists[e])
            lists[e].append(t)
    maxload = max(len(l) for l in lists)
    C = max(32, -(-maxload // 8) * 8)
    assert C <= 128, f"expert overload {maxload}: splitting not implemented"

    xT = np.ascontiguousarray(x.T)  # fp32 [H, T]

    w_gate_up = np.asarray(w_gate_up, dtype=np.float32)
    w_down = np.asarray(w_down, dtype=np.float32)
    ws_gate_up = np.asarray(ws_gate_up, dtype=np.float32)
    ws_down = np.asarray(ws_down, dtype=np.float32)

    # ---- fp8 weight quantization + stationary-tile layout (all experts) ----
    # gate/up tile (pair m, which, k): [h_sub p, f_col c] = W[w*FFN+m*128+c, k*128+p]
    gu8 = _quant8(w_gate_up)  # [E, 2816, 2048]
    gu_l = np.ascontiguousarray(
        gu8.reshape(N_EXPERTS, 2, N_MF, 128, N_KH, 128)
        .transpose(0, 5, 2, 1, 4, 3)
        .reshape(N_EXPERTS, 128, GU_FREE)
    )
    # down tile (hm, j): [f_sub p, h_col c] = W[hm*128+c, j*128+p]
    dn8 = _quant8(w_down)  # [E, 2048, 1408]
    dn_l = np.ascontiguousarray(
        dn8.reshape(N_EXPERTS, N_MH, 128, N_MF, 128)
        .transpose(0, 4, 1, 3, 2)
        .reshape(N_EXPERTS, 128, DN_FREE)
    )
    w8_all = np.concatenate([gu_l, dn_l], axis=2)  # [E, 128, GU_FREE+DN_FREE]

    xT_s = xT / S8  # tokens pre-scaled so gate/up PSUM is exact (power of 2)

    in_maps = []
    for c in range(N_CORES):
        xts = np.zeros((EPC, 128, N_KH * C), dtype=BF16)
        for s in range(EPC):
            e = c * EPC + s
            toks = lists[e]
            n = len(toks)
            if n:
                xte = np.zeros((HIDDEN, C), dtype=np.float32)
                xte[:, :n] = xT_s[:, toks]
                xts[s] = _sbufize(xte.astype(BF16), N_KH)
        # shared expert slice (tensor-parallel on intermediate dim), padded to
        # 3x128 cols of zeros beyond 352 so every stationary tile is 128 wide
        g_pad = np.zeros((N_SF * 128, HIDDEN), dtype=np.float32)
        u_pad = np.zeros((N_SF * 128, HIDDEN), dtype=np.float32)
        g_pad[:SFS] = ws_gate_up[c * SFS:(c + 1) * SFS]
        u_pad[:SFS] = ws_gate_up[SHARED_FFN + c * SFS: SHARED_FFN + (c + 1) * SFS]
        gu_sh = np.concatenate([g_pad, u_pad], axis=0)       # [768, H] (f, h)
        # tile (mt, k): [h_sub p, f_col c] = gu_sh[mt*128+c, k*128+p]
        wsgu = np.ascontiguousarray(
            gu_sh.reshape(2 * N_SF, 128, N_KH, 128)
            .transpose(3, 0, 2, 1)
            .reshape(128, 2 * N_SF * N_KH * 128)
            .astype(BF16)
        )
        wsd_pad = np.zeros((N_SF * 128, HIDDEN), dtype=np.float32)
        wsd_pad[:SFS] = ws_down[:, c * SFS:(c + 1) * SFS].T  # [384, H] (f, h)
        # tile (hm, j): [f_sub p, h_col c] = wsd_pad[j*128+p, hm*128+c]
        wsd = np.ascontiguousarray(
            wsd_pad.reshape(N_SF, 128, N_MH, 128)
            .transpose(1, 2, 0, 3)
            .reshape(128, N_MH * N_SF * 128)
            .astype(BF16)
        )
        xsh = _sbufize(xT.astype(BF16), N_KH)               # [128, 16*256]
        in_maps.append({
            "w8": w8_all[c * EPC:(c + 1) * EPC], "xt": xts,
            "xsh": xsh, "wsgu": wsgu, "wsd": wsd,
        })

    if C not in _PROGRAM_CACHE:
        _PROGRAM_CACHE[C] = _build_program(C)
    nc = _PROGRAM_CACHE[C]

    from concourse.bass_utils import run_bass_kernel_spmd
    res = run_bass_kernel_spmd(
        nc, in_maps, list(range(N_CORES)),
        trace=bool(os.environ.get("MOE_KERNEL_TRACE")),
    )
    LAST_RESULTS = res

    # ---- combine: weighted gather-sum of routed rows + shared partials ----
    # yrt[s] is [128, N_MH*C] with y[e, tok c, hm*128+p] = buf[p, hm*C + c]
    y_all = np.stack([
        r["yrt"].astype(np.float32).reshape(EPC, 128, N_MH, C)
        for r in res.results
    ])                                                      # [8, EPC, 128, 16, C]
    y_flat = y_all.transpose(0, 1, 4, 3, 2).reshape(N_EXPERTS * C, HIDDEN)
    G = ids * C + pos                                       # [T, 6]
    routed = (y_flat[G] * tw[:, :, None]).sum(axis=1) / S8
    # ysh is [128, N_MH*256] with y[tok, hm*128+p] = buf[p, hm*256 + tok]
    shared = np.sum([
        r["ysh"].astype(np.float32).reshape(128, N_MH, 256).transpose(2, 1, 0)
        for r in res.results
    ], axis=0).reshape(T, HIDDEN)
    out = routed + shared
    return out.reshape(1, T, HIDDEN).astype(np.float32)
